# revision 6
# baseline (speedup 1.0000x reference)
"""GraphSAGE/GCN scoring model as a distributed Bass kernel on 8 TRN2 NeuronCores.

Strategy (graph-parallel, per sharding hint):
  - Nodes partitioned contiguously across 8 cores (6250 each, padded to 6272).
  - Layer transforms (X@W1, h1@W2) computed on the owning core only; the small
    transformed features (t: [N,128] bf16, t2: [N,64] bf16) are AllGathered so
    every core can gather arbitrary src rows for its edge partition.
  - Message passing: edges sorted by dst, grouped into 128-node dst blocks;
    per 128-edge tile, a one-hot selection matrix S[e,d] = norm[e]*(dst_local[e]==d)
    is built on the vector engine and the segment-sum becomes S^T @ gathered_rows
    on the tensor engine (PSUM accumulation across a block's edge tiles).
  - Self-loops handled analytically: out[i] += dis[i]^2 * t[i] (no gather).
  - Score network replicated on every core (cheap); each core scores the
    candidates living in its node range; host sums the 8 partial score arrays.
"""
import sys

sys.path.insert(0, "/opt/trn_rl_repo")

import numpy as np
import ml_dtypes
from contextlib import ExitStack

import concourse.bass as bass
import concourse.tile as tile
from concourse import bacc, mybir
from concourse.bass_utils import run_bass_kernel_spmd
from concourse.masks import make_identity

P = 128
NC = 8
N_NODES = 50000
NPC = N_NODES // NC          # 6250 real nodes per core
NB = (NPC + P - 1) // P      # 49 dst blocks per core
NPCP = NB * P                # 6272 padded nodes per core
NAG = NC * NPCP              # 50176 rows in all-gathered buffers
F_IN = 768
H = 128
D = 64
BQ = 256
KC = 20
BN_EPS = 1e-5

F32 = mybir.dt.float32
BF16 = mybir.dt.bfloat16
I32 = mybir.dt.int32

_CACHE = {}
_RUN_KW = {}   # extra kwargs for run_bass_kernel_spmd (test harness: trace=True etc.)
_LAST = [None]  # last BassKernelResults, for the test harness
_LAST_INMAPS = None  # last per-core input maps, for the bench harness


def _build(tmax: int):
    """Build the SPMD bass program (same program for all 8 cores)."""
    nc = bacc.Bacc("TRN2", target_bir_lowering=False, debug=False, num_devices=NC)

    # ---------------- inputs ----------------
    xp = nc.dram_tensor("xp", [NPCP, F_IN], F32, kind="ExternalInput")
    w1 = nc.dram_tensor("w1", [F_IN, H], F32, kind="ExternalInput")
    w2 = nc.dram_tensor("w2", [H, D], F32, kind="ExternalInput")
    b1 = nc.dram_tensor("b1", [1, H], F32, kind="ExternalInput")
    b2 = nc.dram_tensor("b2", [1, D], F32, kind="ExternalInput")
    query = nc.dram_tensor("query", [BQ, F_IN], F32, kind="ExternalInput")
    snw1 = nc.dram_tensor("snw1", [F_IN, 256], F32, kind="ExternalInput")
    snw2 = nc.dram_tensor("snw2", [256, D], F32, kind="ExternalInput")
    snb1 = nc.dram_tensor("snb1", [1, 256], F32, kind="ExternalInput")
    snb2 = nc.dram_tensor("snb2", [1, D], F32, kind="ExternalInput")
    bng = nc.dram_tensor("bng", [1, 256], F32, kind="ExternalInput")
    bnb = nc.dram_tensor("bnb", [1, 256], F32, kind="ExternalInput")
    bnm = nc.dram_tensor("bnm", [1, 256], F32, kind="ExternalInput")
    bnv = nc.dram_tensor("bnv", [1, 256], F32, kind="ExternalInput")
    esrc = nc.dram_tensor("esrc", [NB, P, tmax], I32, kind="ExternalInput")
    edstl = nc.dram_tensor("edstl", [NB, P, tmax], BF16, kind="ExternalInput")
    enorm = nc.dram_tensor("enorm", [NB, P, tmax], BF16, kind="ExternalInput")
    dsqt = nc.dram_tensor("dsqt", [P, NB], F32, kind="ExternalInput")
    cloc = nc.dram_tensor("cloc", [2, P, KC], I32, kind="ExternalInput")
    cmask = nc.dram_tensor("cmask", [2, P, KC], F32, kind="ExternalInput")

    # ---------------- outputs ----------------
    score = nc.dram_tensor("score", [2, P, KC], F32, kind="ExternalOutput")

    # ---------------- internal DRAM ----------------
    ag1_in = nc.dram_tensor("ag1_in", [NPCP, H], BF16)
    ag1_out = nc.dram_tensor("ag1_out", [NAG, H], BF16, addr_space="Shared")
    ag2_in = nc.dram_tensor("ag2_in", [NPCP, D], BF16)
    ag2_out = nc.dram_tensor("ag2_out", [NAG, D], BF16, addr_space="Shared")
    ng_p = nc.dram_tensor("ng_p", [NPCP, D], F32)

    KT = F_IN // P  # 6 contraction tiles for F_IN

    with tile.TileContext(nc) as tc, ExitStack() as ctx:
        const = ctx.enter_context(tc.tile_pool(name="const", bufs=1))
        tp_pool = ctx.enter_context(tc.tile_pool(name="tp", bufs=3))        # t blocks (persistent)
        t2p_pool = ctx.enter_context(tc.tile_pool(name="t2p", bufs=3))
        xload = ctx.enter_context(tc.tile_pool(name="xload", bufs=3))
        work = ctx.enter_context(tc.tile_pool(name="work", bufs=3))
        gpool = ctx.enter_context(tc.tile_pool(name="gpool", bufs=3))
        spool = ctx.enter_context(tc.tile_pool(name="spool", bufs=3))
        psA = ctx.enter_context(tc.tile_pool(name="psA", bufs=2, space="PSUM"))
        psT = ctx.enter_context(tc.tile_pool(name="psT", bufs=2, space="PSUM"))

        # ---- constants in SBUF
        ident = const.tile([P, P], BF16)
        make_identity(nc, ident[:])

        w1_sb = const.tile([P, KT * H], BF16)
        for k in range(KT):
            w1f = work.tile([P, H], F32, tag="wload")
            nc.sync.dma_start(w1f[:], w1[k * P:(k + 1) * P, :])
            nc.vector.tensor_copy(out=w1_sb[:, k * H:(k + 1) * H], in_=w1f[:])
        w2_sb = const.tile([P, D], BF16)
        w2f = work.tile([P, D], F32, tag="wload2")
        nc.sync.dma_start(w2f[:], w2[:])
        nc.vector.tensor_copy(out=w2_sb[:], in_=w2f[:])

        b1_row = const.tile([1, H], F32)
        nc.sync.dma_start(b1_row[:], b1[:])
        b1_bc = const.tile([P, H], F32)
        nc.gpsimd.partition_broadcast(b1_bc[:], b1_row[:])
        b2_row = const.tile([1, D], F32)
        nc.sync.dma_start(b2_row[:], b2[:])
        b2_bc = const.tile([P, D], F32)
        nc.gpsimd.partition_broadcast(b2_bc[:], b2_row[:])

        dsq_sb = const.tile([P, NB], F32)
        nc.sync.dma_start(dsq_sb[:], dsqt[:])

        iota_big = const.tile([P, tmax, P], BF16)
        nc.gpsimd.iota(
            iota_big[:], pattern=[[0, tmax], [1, P]], base=0,
            channel_multiplier=0, allow_small_or_imprecise_dtypes=True,
        )

        # persistent t / t2 blocks
        t_blks = [tp_pool.tile([P, H], BF16, tag=f"tb{b}", name=f"tb{b}") for b in range(NB)]
        t2_blks = [t2p_pool.tile([P, D], BF16, tag=f"t2b{b}", name=f"t2b{b}") for b in range(NB)]

        # =========== score network (replicated; no cross-core deps) ===========
        # BN folded constants: a = gamma / sqrt(var+eps); c = (b1 - mean)*a + beta
        bn_a = const.tile([1, 256], F32)
        bn_c = const.tile([1, 256], F32)
        tmp1 = work.tile([1, 256], F32, tag="bn")
        tmp2 = work.tile([1, 256], F32, tag="bn2")
        nc.sync.dma_start(tmp1[:], bnv[:])
        nc.vector.tensor_scalar(out=tmp1[:], in0=tmp1[:], scalar1=BN_EPS, scalar2=None,
                                op0=mybir.AluOpType.add)
        nc.vector.reciprocal(tmp1[:], tmp1[:])
        nc.scalar.activation(tmp1[:], tmp1[:], mybir.ActivationFunctionType.Sqrt)
        nc.sync.dma_start(tmp2[:], bng[:])
        nc.vector.tensor_tensor(out=bn_a[:], in0=tmp1[:], in1=tmp2[:],
                                op=mybir.AluOpType.mult)
        nc.sync.dma_start(tmp1[:], snb1[:])
        nc.sync.dma_start(tmp2[:], bnm[:])
        nc.vector.tensor_tensor(out=tmp1[:], in0=tmp1[:], in1=tmp2[:],
                                op=mybir.AluOpType.subtract)
        nc.vector.tensor_tensor(out=tmp1[:], in0=tmp1[:], in1=bn_a[:],
                                op=mybir.AluOpType.mult)
        nc.sync.dma_start(tmp2[:], bnb[:])
        nc.vector.tensor_tensor(out=bn_c[:], in0=tmp1[:], in1=tmp2[:],
                                op=mybir.AluOpType.add)
        bn_a_bc = const.tile([P, 256], F32)
        nc.gpsimd.partition_broadcast(bn_a_bc[:], bn_a[:])
        bn_c_bc = const.tile([P, 256], F32)
        nc.gpsimd.partition_broadcast(bn_c_bc[:], bn_c[:])
        snb2_row = const.tile([1, D], F32)
        nc.sync.dma_start(snb2_row[:], snb2[:])
        snb2_bc = const.tile([P, D], F32)
        nc.gpsimd.partition_broadcast(snb2_bc[:], snb2_row[:])

        snw1_sb = const.tile([P, KT * 256], BF16)
        for k in range(KT):
            wf = work.tile([P, 256], F32, tag="wload3")
            nc.sync.dma_start(wf[:], snw1[k * P:(k + 1) * P, :])
            nc.vector.tensor_copy(out=snw1_sb[:, k * 256:(k + 1) * 256], in_=wf[:])
        snw2_sb = const.tile([P, 2 * D], BF16)
        for k in range(2):
            wf = work.tile([P, D], F32, tag="wload4")
            nc.sync.dma_start(wf[:], snw2[k * P:(k + 1) * P, :])
            nc.vector.tensor_copy(out=snw2_sb[:, k * D:(k + 1) * D], in_=wf[:])

        q_tiles = []
        for rt in range(2):
            qf = work.tile([P, F_IN], F32, tag="qload")
            nc.sync.dma_start(qf[:], query[rt * P:(rt + 1) * P, :])
            qb = work.tile([P, F_IN], BF16, tag="qcast")
            nc.vector.tensor_copy(out=qb[:], in_=qf[:])
            z_ps = psA.tile([P, 256], F32, tag="acc")
            for k in range(KT):
                qt_ps = psT.tile([P, P], BF16, tag="tp")
                nc.tensor.transpose(out=qt_ps[:], in_=qb[:, k * P:(k + 1) * P],
                                    identity=ident[:])
                qt_sb = work.tile([P, P], BF16, tag="qT")
                nc.vector.tensor_copy(out=qt_sb[:], in_=qt_ps[:])
                nc.tensor.matmul(z_ps[:], lhsT=qt_sb[:], rhs=snw1_sb[:, k * 256:(k + 1) * 256],
                                 start=(k == 0), stop=(k == KT - 1))
            z_sb = work.tile([P, 256], F32, tag="zsb")
            nc.vector.tensor_tensor(out=z_sb[:], in0=z_ps[:], in1=bn_a_bc[:],
                                    op=mybir.AluOpType.mult)
            nc.vector.tensor_tensor(out=z_sb[:], in0=z_sb[:], in1=bn_c_bc[:],
                                    op=mybir.AluOpType.add)
            z_bf = work.tile([P, 256], BF16, tag="zbf")
            nc.scalar.activation(z_bf[:], z_sb[:], mybir.ActivationFunctionType.Sigmoid)
            q_ps = psA.tile([P, D], F32, tag="acc")
            for k in range(2):
                zt_ps = psT.tile([P, P], BF16, tag="tp")
                nc.tensor.transpose(out=zt_ps[:], in_=z_bf[:, k * P:(k + 1) * P],
                                    identity=ident[:])
                zt_sb = work.tile([P, P], BF16, tag="zT")
                nc.vector.tensor_copy(out=zt_sb[:], in_=zt_ps[:])
                nc.tensor.matmul(q_ps[:], lhsT=zt_sb[:], rhs=snw2_sb[:, k * D:(k + 1) * D],
                                 start=(k == 0), stop=(k == 1))
            q_sb = const.tile([P, D], F32, tag=f"q{rt}")
            nc.vector.tensor_tensor(out=q_sb[:], in0=q_ps[:], in1=snb2_bc[:],
                                    op=mybir.AluOpType.add)
            q_tiles.append(q_sb)

        # =========== node transform: t = Xp @ W1  (per 128-row block) ===========
        for bidx in range(NB):
            xf = xload.tile([P, F_IN], F32, tag="xf")
            nc.sync.dma_start(xf[:], xp[bidx * P:(bidx + 1) * P, :])
            xb = xload.tile([P, F_IN], BF16, tag="xb")
            nc.vector.tensor_copy(out=xb[:], in_=xf[:])
            acc = psA.tile([P, H], F32, tag="acc")
            for k in range(KT):
                xt_ps = psT.tile([P, P], BF16, tag="tp")
                nc.tensor.transpose(out=xt_ps[:], in_=xb[:, k * P:(k + 1) * P],
                                    identity=ident[:])
                xt_sb = work.tile([P, P], BF16, tag="xT")
                nc.vector.tensor_copy(out=xt_sb[:], in_=xt_ps[:])
                nc.tensor.matmul(acc[:], lhsT=xt_sb[:], rhs=w1_sb[:, k * H:(k + 1) * H],
                                 start=(k == 0), stop=(k == KT - 1))
            nc.vector.tensor_copy(out=t_blks[bidx][:], in_=acc[:])
            nc.sync.dma_start(ag1_in[bidx * P:(bidx + 1) * P, :], t_blks[bidx][:])

        # =========== AllGather t ===========
        nc.gpsimd.collective_compute(
            "AllGather", mybir.AluOpType.bypass,
            replica_groups=[list(range(NC))],
            ins=[ag1_in[:].opt()], outs=[ag1_out[:].opt()],
        )

        # =========== GCN layer 1 + transform 2 ===========
        for bidx in range(NB):
            dstl_sb = spool.tile([P, tmax], BF16, tag="dstl")
            nc.sync.dma_start(dstl_sb[:], edstl[bidx])
            norm_sb = spool.tile([P, tmax], BF16, tag="norm")
            nc.sync.dma_start(norm_sb[:], enorm[bidx])
            src_sb = spool.tile([P, tmax], I32, tag="srcs")
            nc.sync.dma_start(src_sb[:], esrc[bidx])

            s_t = spool.tile([P, tmax, P], BF16, tag="s_t")
            nc.vector.tensor_tensor(
                out=s_t[:], in0=iota_big[:],
                in1=dstl_sb[:].unsqueeze(-1).to_broadcast([P, tmax, P]),
                op=mybir.AluOpType.is_equal)
            nc.vector.tensor_tensor(
                out=s_t[:], in0=s_t[:],
                in1=norm_sb[:].unsqueeze(-1).to_broadcast([P, tmax, P]),
                op=mybir.AluOpType.mult)

            g = gpool.tile([P, tmax, H], BF16, tag="g1")
            acc = psA.tile([P, H], F32, tag="acc")
            for t in range(tmax):
                nc.gpsimd.indirect_dma_start(
                    out=g[:, t, :], out_offset=None, in_=ag1_out[:],
                    in_offset=bass.IndirectOffsetOnAxis(ap=src_sb[:, t:t + 1], axis=0),
                )
                nc.tensor.matmul(acc[:], lhsT=s_t[:, t, :], rhs=g[:, t, :],
                                 start=(t == 0), stop=(t == tmax - 1))
            # self-loop + bias, sigmoid
            selft = work.tile([P, H], F32, tag="self1")
            nc.vector.tensor_scalar(out=selft[:], in0=t_blks[bidx][:],
                                    scalar1=dsq_sb[:, bidx:bidx + 1], scalar2=None,
                                    op0=mybir.AluOpType.mult)
            hpre = work.tile([P, H], F32, tag="hpre")
            nc.vector.tensor_tensor(out=hpre[:], in0=acc[:], in1=selft[:],
                                    op=mybir.AluOpType.add)
            nc.vector.tensor_tensor(out=hpre[:], in0=hpre[:], in1=b1_bc[:],
                                    op=mybir.AluOpType.add)
            h1 = work.tile([P, H], BF16, tag="h1")
            nc.scalar.activation(h1[:], hpre[:], mybir.ActivationFunctionType.Sigmoid)
            # t2 = h1 @ W2
            h1t_ps = psT.tile([P, P], BF16, tag="tp")
            nc.tensor.transpose(out=h1t_ps[:], in_=h1[:], identity=ident[:])
            h1t = work.tile([P, P], BF16, tag="h1T")
            nc.vector.tensor_copy(out=h1t[:], in_=h1t_ps[:])
            t2_ps = psA.tile([P, D], F32, tag="acc")
            nc.tensor.matmul(t2_ps[:], lhsT=h1t[:], rhs=w2_sb[:], start=True, stop=True)
            nc.vector.tensor_copy(out=t2_blks[bidx][:], in_=t2_ps[:])
            nc.sync.dma_start(ag2_in[bidx * P:(bidx + 1) * P, :], t2_blks[bidx][:])

        # =========== AllGather t2 ===========
        nc.gpsimd.collective_compute(
            "AllGather", mybir.AluOpType.bypass,
            replica_groups=[list(range(NC))],
            ins=[ag2_in[:].opt()], outs=[ag2_out[:].opt()],
        )

        # =========== GCN layer 2 ===========
        for bidx in range(NB):
            dstl_sb = spool.tile([P, tmax], BF16, tag="dstl")
            nc.sync.dma_start(dstl_sb[:], edstl[bidx])
            norm_sb = spool.tile([P, tmax], BF16, tag="norm")
            nc.sync.dma_start(norm_sb[:], enorm[bidx])
            src_sb = spool.tile([P, tmax], I32, tag="srcs")
            nc.sync.dma_start(src_sb[:], esrc[bidx])

            s_t = spool.tile([P, tmax, P], BF16, tag="s_t")
            nc.vector.tensor_tensor(
                out=s_t[:], in0=iota_big[:],
                in1=dstl_sb[:].unsqueeze(-1).to_broadcast([P, tmax, P]),
                op=mybir.AluOpType.is_equal)
            nc.vector.tensor_tensor(
                out=s_t[:], in0=s_t[:],
                in1=norm_sb[:].unsqueeze(-1).to_broadcast([P, tmax, P]),
                op=mybir.AluOpType.mult)

            g = gpool.tile([P, tmax, D], BF16, tag="g2")
            acc = psA.tile([P, D], F32, tag="acc")
            for t in range(tmax):
                nc.gpsimd.indirect_dma_start(
                    out=g[:, t, :], out_offset=None, in_=ag2_out[:],
                    in_offset=bass.IndirectOffsetOnAxis(ap=src_sb[:, t:t + 1], axis=0),
                )
                nc.tensor.matmul(acc[:], lhsT=s_t[:, t, :], rhs=g[:, t, :],
                                 start=(t == 0), stop=(t == tmax - 1))
            selft = work.tile([P, D], F32, tag="self2")
            nc.vector.tensor_scalar(out=selft[:], in0=t2_blks[bidx][:],
                                    scalar1=dsq_sb[:, bidx:bidx + 1], scalar2=None,
                                    op0=mybir.AluOpType.mult)
            ng = work.tile([P, D], F32, tag="ng")
            nc.vector.tensor_tensor(out=ng[:], in0=acc[:], in1=selft[:],
                                    op=mybir.AluOpType.add)
            nc.vector.tensor_tensor(out=ng[:], in0=ng[:], in1=b2_bc[:],
                                    op=mybir.AluOpType.add)
            nc.sync.dma_start(ng_p[bidx * P:(bidx + 1) * P, :], ng[:])

        # =========== candidate scoring ===========
        for rt in range(2):
            cl_sb = work.tile([P, KC], I32, tag="cl")
            nc.sync.dma_start(cl_sb[:], cloc[rt])
            cm_sb = work.tile([P, KC], F32, tag="cm")
            nc.sync.dma_start(cm_sb[:], cmask[rt])
            sc_sb = work.tile([P, KC], F32, tag="sc")
            for k in range(KC):
                cg = gpool.tile([P, D], F32, tag="cg")
                nc.gpsimd.indirect_dma_start(
                    out=cg[:], out_offset=None, in_=ng_p[:],
                    in_offset=bass.IndirectOffsetOnAxis(ap=cl_sb[:, k:k + 1], axis=0),
                )
                prod = work.tile([P, D], F32, tag="prod")
                nc.vector.tensor_tensor(out=prod[:], in0=cg[:], in1=q_tiles[rt][:],
                                        op=mybir.AluOpType.mult)
                nc.vector.tensor_reduce(out=sc_sb[:, k:k + 1], in_=prod[:],
                                        axis=mybir.AxisListType.X,
                                        op=mybir.AluOpType.add)
            nc.vector.tensor_tensor(out=sc_sb[:], in0=sc_sb[:], in1=cm_sb[:],
                                    op=mybir.AluOpType.mult)
            nc.sync.dma_start(score[rt], sc_sb[:])

    nc.compile()
    return nc


def _prep(edge_index: np.ndarray):
    """Host-side graph preprocessing: degrees, norms, per-core sorted edge tiles."""
    src = edge_index[0].astype(np.int64)
    dst = edge_index[1].astype(np.int64)
    deg = np.bincount(dst, minlength=N_NODES).astype(np.float64) + 1.0
    dis = 1.0 / np.sqrt(deg)
    norm = (dis[src] * dis[dst]).astype(np.float32)
    dsq = (dis * dis).astype(np.float32)

    owner = dst // NPC
    order = np.argsort(dst, kind="stable")
    src_s, dst_s, norm_s, own_s = src[order], dst[order], norm[order], owner[order]
    # AG row remap for src
    srow = (src_s // NPC) * NPCP + (src_s % NPC)

    # per (core, block) counts
    local = dst_s - own_s * NPC
    blk = local // P
    dstl = (local % P).astype(np.float32)
    key = own_s * NB + blk
    counts = np.bincount(key, minlength=NC * NB).reshape(NC, NB)
    tmax = int(np.ceil(counts.max() / P))

    E_pad = NB * tmax * P
    esrc = np.zeros((NC, NB, tmax * P), np.int32)
    edstl = np.zeros((NC, NB, tmax * P), np.float32)
    enorm = np.zeros((NC, NB, tmax * P), np.float32)
    # edges are already sorted by (owner, blk) since dst sorted & partitions contiguous
    starts = np.zeros(NC * NB + 1, np.int64)
    np.cumsum(counts.reshape(-1), out=starts[1:])
    for c in range(NC):
        for b in range(NB):
            s, e = starts[c * NB + b], starts[c * NB + b + 1]
            n = e - s
            esrc[c, b, :n] = srow[s:e]
            edstl[c, b, :n] = dstl[s:e]
            enorm[c, b, :n] = norm_s[s:e]
    # transpose inner to [NB, P, tmax]: element (b, t*P+p) -> (b, p, t)
    esrc = esrc.reshape(NC, NB, tmax, P).transpose(0, 1, 3, 2).copy()
    edstl = edstl.reshape(NC, NB, tmax, P).transpose(0, 1, 3, 2)
    enorm = enorm.reshape(NC, NB, tmax, P).transpose(0, 1, 3, 2)
    edstl = edstl.astype(ml_dtypes.bfloat16)
    enorm = enorm.astype(ml_dtypes.bfloat16)

    # dsq transposed per core: [P, NB], entry (p, b) = dsq[c*NPC + b*P + p] (0 for pads)
    dsqt = np.zeros((NC, P, NB), np.float32)
    for c in range(NC):
        v = np.zeros(NPCP, np.float32)
        v[:NPC] = dsq[c * NPC:(c + 1) * NPC]
        dsqt[c] = v.reshape(NB, P).T
    return esrc, edstl, enorm, dsqt, tmax


def kernel(query_embedding, names_embedding, w1, b1, w2, b2,
           sn_w1, sn_b1, bn_gamma, bn_beta, bn_mean, bn_var,
           sn_w2, sn_b2, edge_index, candidates_indices, top_k):
    query_embedding = np.asarray(query_embedding, np.float32)
    names_embedding = np.asarray(names_embedding, np.float32)
    edge_index = np.asarray(edge_index)
    candidates_indices = np.asarray(candidates_indices)

    esrc, edstl, enorm, dsqt, tmax = _prep(edge_index)

    if tmax not in _CACHE:
        _CACHE[tmax] = _build(tmax)
    nc = _CACHE[tmax]

    # candidate partitioning
    cand = candidates_indices.astype(np.int64)  # [256, 20]
    cown = cand // NPC
    clocal = (cand % NPC).astype(np.int32)

    in_maps = []
    for c in range(NC):
        xp = np.zeros((NPCP, F_IN), np.float32)
        xp[:NPC] = names_embedding[c * NPC:(c + 1) * NPC]
        cl = np.where(cown == c, clocal, 0).astype(np.int32).reshape(2, P, KC)
        cm = (cown == c).astype(np.float32).reshape(2, P, KC)
        in_maps.append(dict(
            xp=xp, w1=np.asarray(w1, np.float32), w2=np.asarray(w2, np.float32),
            b1=np.asarray(b1, np.float32).reshape(1, H),
            b2=np.asarray(b2, np.float32).reshape(1, D),
            query=query_embedding,
            snw1=np.asarray(sn_w1, np.float32), snw2=np.asarray(sn_w2, np.float32),
            snb1=np.asarray(sn_b1, np.float32).reshape(1, 256),
            snb2=np.asarray(sn_b2, np.float32).reshape(1, D),
            bng=np.asarray(bn_gamma, np.float32).reshape(1, 256),
            bnb=np.asarray(bn_beta, np.float32).reshape(1, 256),
            bnm=np.asarray(bn_mean, np.float32).reshape(1, 256),
            bnv=np.asarray(bn_var, np.float32).reshape(1, 256),
            esrc=esrc[c], edstl=edstl[c], enorm=enorm[c], dsqt=dsqt[c],
            cloc=cl, cmask=cm,
        ))

    global _LAST_INMAPS
    _LAST_INMAPS = in_maps
    res = run_bass_kernel_spmd(nc, in_maps, core_ids=list(range(NC)), **_RUN_KW)
    _LAST[0] = res
    out = np.zeros((BQ, KC), np.float32)
    for c in range(NC):
        out += res.results[c]["score"].reshape(BQ, KC)

    k = int(top_k) if np.ndim(top_k) == 0 else int(np.asarray(top_k).item())
    return out[:, :k].copy() if k != KC else out


# revision 11
# speedup vs baseline: 19.5480x; 19.5480x over previous
"""GraphSAGE/GCN scoring model as a distributed Bass kernel on 8 TRN2 NeuronCores.

Strategy (graph-parallel, per sharding hint):
  - Nodes partitioned contiguously across 8 cores (6250 each, padded to 6272).
  - Layer transforms (X@W1, h1@W2) computed on the owning core only; the small
    transformed features (t: [N,128] bf16, t2: [N,64] bf16) are AllGathered so
    every core can gather arbitrary src rows for its edge partition.
  - Message passing: edges sorted by dst, grouped into 128-node dst blocks;
    per 128-edge tile, a one-hot selection matrix S[e,d] = norm[e]*(dst_local[e]==d)
    is built on the vector engine and the segment-sum becomes S^T @ gathered_rows
    on the tensor engine (PSUM accumulation across a block's edge tiles).
  - Self-loops handled analytically: out[i] += dis[i]^2 * t[i] (no gather).
  - Score network replicated on every core (cheap); each core scores the
    candidates living in its node range; host sums the 8 partial score arrays.
"""
import sys

sys.path.insert(0, "/opt/trn_rl_repo")

import numpy as np
import ml_dtypes
from contextlib import ExitStack

import concourse.bass as bass
import concourse.tile as tile
from concourse import bacc, mybir
from concourse.bass_utils import run_bass_kernel_spmd
from concourse.masks import make_identity

P = 128
NC = 8
N_NODES = 50000
NPC = N_NODES // NC          # 6250 real nodes per core
NB = (NPC + P - 1) // P      # 49 dst blocks per core
NPCP = NB * P                # 6272 padded nodes per core
NAG = NC * NPCP              # 50176 rows in all-gathered buffers
F_IN = 768
H = 128
D = 64
BQ = 256
KC = 20
BN_EPS = 1e-5
I16_SPLIT = 32768  # int16 index ceiling for dma_gather; high rows gathered from an offset view

F32 = mybir.dt.float32
BF16 = mybir.dt.bfloat16
I32 = mybir.dt.int32

_CACHE = {}
_RUN_KW = {}   # extra kwargs for run_bass_kernel_spmd (test harness: trace=True etc.)
_LAST = [None]  # last BassKernelResults, for the test harness
_LAST_INMAPS = None  # last per-core input maps, for the bench harness


def _build(tbs: tuple, single: bool = False):
    """Build the SPMD bass program (same program for all 8 cores)."""
    nc = bacc.Bacc("TRN2", target_bir_lowering=False, debug=False, num_devices=NC)

    # ---------------- inputs ----------------
    xp = nc.dram_tensor("xp", [NPCP, F_IN], F32, kind="ExternalInput")
    w1 = nc.dram_tensor("w1", [F_IN, H], F32, kind="ExternalInput")
    w2 = nc.dram_tensor("w2", [H, D], F32, kind="ExternalInput")
    b1 = nc.dram_tensor("b1", [1, H], F32, kind="ExternalInput")
    b2 = nc.dram_tensor("b2", [1, D], F32, kind="ExternalInput")
    query = nc.dram_tensor("query", [BQ, F_IN], F32, kind="ExternalInput")
    snw1 = nc.dram_tensor("snw1", [F_IN, 256], F32, kind="ExternalInput")
    snw2 = nc.dram_tensor("snw2", [256, D], F32, kind="ExternalInput")
    snb1 = nc.dram_tensor("snb1", [1, 256], F32, kind="ExternalInput")
    snb2 = nc.dram_tensor("snb2", [1, D], F32, kind="ExternalInput")
    bng = nc.dram_tensor("bng", [1, 256], F32, kind="ExternalInput")
    bnb = nc.dram_tensor("bnb", [1, 256], F32, kind="ExternalInput")
    bnm = nc.dram_tensor("bnm", [1, 256], F32, kind="ExternalInput")
    bnv = nc.dram_tensor("bnv", [1, 256], F32, kind="ExternalInput")
    tmax = max(tbs)
    tot = sum(tbs)
    off = [0]
    for tb in tbs:
        off.append(off[-1] + tb)
    esrc = nc.dram_tensor("esrc", [P, tot], I32, kind="ExternalInput")
    edstl = nc.dram_tensor("edstl", [P, tot], BF16, kind="ExternalInput")
    enorm = nc.dram_tensor("enorm", [P, tot], BF16, kind="ExternalInput")
    dsqt = nc.dram_tensor("dsqt", [P, NB], F32, kind="ExternalInput")
    cloc = nc.dram_tensor("cloc", [2, P, KC], I32, kind="ExternalInput")
    cmask = nc.dram_tensor("cmask", [2, P, KC], F32, kind="ExternalInput")

    # ---------------- outputs ----------------
    score = nc.dram_tensor("score", [2, P, KC], F32, kind="ExternalOutput")

    # ---------------- internal DRAM ----------------
    ag1_in = nc.dram_tensor("ag1_in", [NPCP, H], BF16)
    ag1_out = nc.dram_tensor("ag1_out", [NAG, H], BF16, addr_space="Shared")
    ag2_in = nc.dram_tensor("ag2_in", [NPCP, D], BF16)
    ag2_out = nc.dram_tensor("ag2_out", [NAG, D], BF16, addr_space="Shared")
    ng_p = nc.dram_tensor("ng_p", [NPCP, D], F32)

    KT = F_IN // P  # 6 contraction tiles for F_IN

    with tile.TileContext(nc) as tc, ExitStack() as ctx:
        const = ctx.enter_context(tc.tile_pool(name="const", bufs=1))
        tp_pool = ctx.enter_context(tc.tile_pool(name="tp", bufs=3))        # t blocks (persistent)
        t2p_pool = ctx.enter_context(tc.tile_pool(name="t2p", bufs=3))
        xload = ctx.enter_context(tc.tile_pool(name="xload", bufs=3))
        work = ctx.enter_context(tc.tile_pool(name="work", bufs=3))
        gpool = ctx.enter_context(tc.tile_pool(name="gpool", bufs=3))
        spool = ctx.enter_context(tc.tile_pool(name="spool", bufs=3))
        psA = ctx.enter_context(tc.tile_pool(name="psA", bufs=2, space="PSUM"))
        psT = ctx.enter_context(tc.tile_pool(name="psT", bufs=2, space="PSUM"))

        # ---- constants in SBUF
        ident = const.tile([P, P], BF16)
        make_identity(nc, ident[:])

        w1_sb = const.tile([P, KT * H], BF16)
        for k in range(KT):
            w1f = work.tile([P, H], F32, tag="wload")
            nc.sync.dma_start(w1f[:], w1[k * P:(k + 1) * P, :])
            nc.vector.tensor_copy(out=w1_sb[:, k * H:(k + 1) * H], in_=w1f[:])
        w2_sb = const.tile([P, D], BF16)
        w2f = work.tile([P, D], F32, tag="wload2")
        nc.sync.dma_start(w2f[:], w2[:])
        nc.vector.tensor_copy(out=w2_sb[:], in_=w2f[:])

        b1_row = const.tile([1, H], F32)
        nc.sync.dma_start(b1_row[:], b1[:])
        b1_bc = const.tile([P, H], F32)
        nc.gpsimd.partition_broadcast(b1_bc[:], b1_row[:])
        b2_row = const.tile([1, D], F32)
        nc.sync.dma_start(b2_row[:], b2[:])
        b2_bc = const.tile([P, D], F32)
        nc.gpsimd.partition_broadcast(b2_bc[:], b2_row[:])

        dsq_sb = const.tile([P, NB], F32)
        nc.sync.dma_start(dsq_sb[:], dsqt[:])

        iota_big = const.tile([P, tmax, P], BF16)
        nc.gpsimd.iota(
            iota_big[:], pattern=[[0, tmax], [1, P]], base=0,
            channel_multiplier=0, allow_small_or_imprecise_dtypes=True,
        )

        # persistent t / t2 blocks
        t_blks = [tp_pool.tile([P, H], BF16, tag=f"tb{b}", name=f"tb{b}") for b in range(NB)]
        t2_blks = [t2p_pool.tile([P, D], BF16, tag=f"t2b{b}", name=f"t2b{b}") for b in range(NB)]

        # =========== score network (replicated; no cross-core deps) ===========
        # BN folded constants: a = gamma / sqrt(var+eps); c = (b1 - mean)*a + beta
        bn_a = const.tile([1, 256], F32)
        bn_c = const.tile([1, 256], F32)
        tmp1 = work.tile([1, 256], F32, tag="bn")
        tmp2 = work.tile([1, 256], F32, tag="bn2")
        nc.sync.dma_start(tmp1[:], bnv[:])
        nc.vector.tensor_scalar(out=tmp1[:], in0=tmp1[:], scalar1=BN_EPS, scalar2=None,
                                op0=mybir.AluOpType.add)
        nc.vector.reciprocal(tmp1[:], tmp1[:])
        nc.scalar.activation(tmp1[:], tmp1[:], mybir.ActivationFunctionType.Sqrt)
        nc.sync.dma_start(tmp2[:], bng[:])
        nc.vector.tensor_tensor(out=bn_a[:], in0=tmp1[:], in1=tmp2[:],
                                op=mybir.AluOpType.mult)
        nc.sync.dma_start(tmp1[:], snb1[:])
        nc.sync.dma_start(tmp2[:], bnm[:])
        nc.vector.tensor_tensor(out=tmp1[:], in0=tmp1[:], in1=tmp2[:],
                                op=mybir.AluOpType.subtract)
        nc.vector.tensor_tensor(out=tmp1[:], in0=tmp1[:], in1=bn_a[:],
                                op=mybir.AluOpType.mult)
        nc.sync.dma_start(tmp2[:], bnb[:])
        nc.vector.tensor_tensor(out=bn_c[:], in0=tmp1[:], in1=tmp2[:],
                                op=mybir.AluOpType.add)
        bn_a_bc = const.tile([P, 256], F32)
        nc.gpsimd.partition_broadcast(bn_a_bc[:], bn_a[:])
        bn_c_bc = const.tile([P, 256], F32)
        nc.gpsimd.partition_broadcast(bn_c_bc[:], bn_c[:])
        snb2_row = const.tile([1, D], F32)
        nc.sync.dma_start(snb2_row[:], snb2[:])
        snb2_bc = const.tile([P, D], F32)
        nc.gpsimd.partition_broadcast(snb2_bc[:], snb2_row[:])

        snw1_sb = const.tile([P, KT * 256], BF16)
        for k in range(KT):
            wf = work.tile([P, 256], F32, tag="wload3")
            nc.sync.dma_start(wf[:], snw1[k * P:(k + 1) * P, :])
            nc.vector.tensor_copy(out=snw1_sb[:, k * 256:(k + 1) * 256], in_=wf[:])
        snw2_sb = const.tile([P, 2 * D], BF16)
        for k in range(2):
            wf = work.tile([P, D], F32, tag="wload4")
            nc.sync.dma_start(wf[:], snw2[k * P:(k + 1) * P, :])
            nc.vector.tensor_copy(out=snw2_sb[:, k * D:(k + 1) * D], in_=wf[:])

        q_tiles = []
        for rt in range(2):
            qf = work.tile([P, F_IN], F32, tag="qload")
            nc.sync.dma_start(qf[:], query[rt * P:(rt + 1) * P, :])
            qb = work.tile([P, F_IN], BF16, tag="qcast")
            nc.vector.tensor_copy(out=qb[:], in_=qf[:])
            z_ps = psA.tile([P, 256], F32, tag="acc")
            for k in range(KT):
                qt_ps = psT.tile([P, P], BF16, tag="tp")
                nc.tensor.transpose(out=qt_ps[:], in_=qb[:, k * P:(k + 1) * P],
                                    identity=ident[:])
                qt_sb = work.tile([P, P], BF16, tag="qT")
                nc.vector.tensor_copy(out=qt_sb[:], in_=qt_ps[:])
                nc.tensor.matmul(z_ps[:], lhsT=qt_sb[:], rhs=snw1_sb[:, k * 256:(k + 1) * 256],
                                 start=(k == 0), stop=(k == KT - 1))
            z_sb = work.tile([P, 256], F32, tag="zsb")
            nc.vector.tensor_tensor(out=z_sb[:], in0=z_ps[:], in1=bn_a_bc[:],
                                    op=mybir.AluOpType.mult)
            nc.vector.tensor_tensor(out=z_sb[:], in0=z_sb[:], in1=bn_c_bc[:],
                                    op=mybir.AluOpType.add)
            z_bf = work.tile([P, 256], BF16, tag="zbf")
            nc.scalar.activation(z_bf[:], z_sb[:], mybir.ActivationFunctionType.Sigmoid)
            q_ps = psA.tile([P, D], F32, tag="acc")
            for k in range(2):
                zt_ps = psT.tile([P, P], BF16, tag="tp")
                nc.tensor.transpose(out=zt_ps[:], in_=z_bf[:, k * P:(k + 1) * P],
                                    identity=ident[:])
                zt_sb = work.tile([P, P], BF16, tag="zT")
                nc.vector.tensor_copy(out=zt_sb[:], in_=zt_ps[:])
                nc.tensor.matmul(q_ps[:], lhsT=zt_sb[:], rhs=snw2_sb[:, k * D:(k + 1) * D],
                                 start=(k == 0), stop=(k == 1))
            q_sb = const.tile([P, D], F32, tag=f"q{rt}")
            nc.vector.tensor_tensor(out=q_sb[:], in0=q_ps[:], in1=snb2_bc[:],
                                    op=mybir.AluOpType.add)
            q_tiles.append(q_sb)

        # =========== node transform: t = Xp @ W1  (per 128-row block) ===========
        for bidx in range(NB):
            xf = xload.tile([P, F_IN], F32, tag="xf")
            nc.sync.dma_start(xf[:], xp[bidx * P:(bidx + 1) * P, :])
            xb = xload.tile([P, F_IN], BF16, tag="xb")
            nc.vector.tensor_copy(out=xb[:], in_=xf[:])
            acc = psA.tile([P, H], F32, tag="acc")
            for k in range(KT):
                xt_ps = psT.tile([P, P], BF16, tag="tp")
                nc.tensor.transpose(out=xt_ps[:], in_=xb[:, k * P:(k + 1) * P],
                                    identity=ident[:])
                xt_sb = work.tile([P, P], BF16, tag="xT")
                nc.vector.tensor_copy(out=xt_sb[:], in_=xt_ps[:])
                nc.tensor.matmul(acc[:], lhsT=xt_sb[:], rhs=w1_sb[:, k * H:(k + 1) * H],
                                 start=(k == 0), stop=(k == KT - 1))
            nc.vector.tensor_copy(out=t_blks[bidx][:], in_=acc[:])
            nc.sync.dma_start(ag1_in[bidx * P:(bidx + 1) * P, :], t_blks[bidx][:])

        # =========== AllGather t ===========
        if single:
            nc.sync.dma_start(ag1_out[0:NPCP, :], ag1_in[:])
        else:
            nc.gpsimd.collective_compute(
                "AllGather", mybir.AluOpType.bypass,
                replica_groups=[list(range(NC))],
                ins=[ag1_in[:].opt()], outs=[ag1_out[:].opt()],
            )

        # =========== GCN layer 1 + transform 2 ===========
        for bidx in range(NB):
            tb = tbs[bidx]
            o0, o1 = off[bidx], off[bidx + 1]
            dstl_sb = spool.tile([P, tmax], BF16, tag="dstl")
            nc.sync.dma_start(dstl_sb[:, 0:tb], edstl[:, o0:o1])
            norm_sb = spool.tile([P, tmax], BF16, tag="norm")
            nc.sync.dma_start(norm_sb[:, 0:tb], enorm[:, o0:o1])
            src_sb = spool.tile([P, tmax], I32, tag="srcs")
            nc.sync.dma_start(src_sb[:, 0:tb], esrc[:, o0:o1])

            s_t = spool.tile([P, tmax, P], BF16, tag="s_t")
            nc.vector.tensor_tensor(
                out=s_t[:, 0:tb, :], in0=iota_big[:, 0:tb, :],
                in1=dstl_sb[:, 0:tb].unsqueeze(-1).to_broadcast([P, tb, P]),
                op=mybir.AluOpType.is_equal)
            nc.vector.tensor_tensor(
                out=s_t[:, 0:tb, :], in0=s_t[:, 0:tb, :],
                in1=norm_sb[:, 0:tb].unsqueeze(-1).to_broadcast([P, tb, P]),
                op=mybir.AluOpType.mult)

            g = gpool.tile([P, tmax, H], BF16, tag="g1")
            acc = psA.tile([P, H], F32, tag="acc")
            for t in range(tb):
                nc.gpsimd.indirect_dma_start(
                    out=g[:, t, :], out_offset=None, in_=ag1_out[:],
                    in_offset=bass.IndirectOffsetOnAxis(ap=src_sb[:, t:t + 1], axis=0),
                )
                nc.tensor.matmul(acc[:], lhsT=s_t[:, t, :], rhs=g[:, t, :],
                                 start=(t == 0), stop=(t == tb - 1))
            # self-loop + bias, sigmoid
            selft = work.tile([P, H], F32, tag="self1")
            nc.vector.tensor_scalar(out=selft[:], in0=t_blks[bidx][:],
                                    scalar1=dsq_sb[:, bidx:bidx + 1], scalar2=None,
                                    op0=mybir.AluOpType.mult)
            hpre = work.tile([P, H], F32, tag="hpre")
            nc.vector.tensor_tensor(out=hpre[:], in0=acc[:], in1=selft[:],
                                    op=mybir.AluOpType.add)
            nc.vector.tensor_tensor(out=hpre[:], in0=hpre[:], in1=b1_bc[:],
                                    op=mybir.AluOpType.add)
            h1 = work.tile([P, H], BF16, tag="h1")
            nc.scalar.activation(h1[:], hpre[:], mybir.ActivationFunctionType.Sigmoid)
            # t2 = h1 @ W2
            h1t_ps = psT.tile([P, P], BF16, tag="tp")
            nc.tensor.transpose(out=h1t_ps[:], in_=h1[:], identity=ident[:])
            h1t = work.tile([P, P], BF16, tag="h1T")
            nc.vector.tensor_copy(out=h1t[:], in_=h1t_ps[:])
            t2_ps = psA.tile([P, D], F32, tag="acc")
            nc.tensor.matmul(t2_ps[:], lhsT=h1t[:], rhs=w2_sb[:], start=True, stop=True)
            nc.vector.tensor_copy(out=t2_blks[bidx][:], in_=t2_ps[:])
            nc.sync.dma_start(ag2_in[bidx * P:(bidx + 1) * P, :], t2_blks[bidx][:])

        # =========== AllGather t2 ===========
        if single:
            nc.sync.dma_start(ag2_out[0:NPCP, :], ag2_in[:])
        else:
            nc.gpsimd.collective_compute(
                "AllGather", mybir.AluOpType.bypass,
                replica_groups=[list(range(NC))],
                ins=[ag2_in[:].opt()], outs=[ag2_out[:].opt()],
            )

        # =========== GCN layer 2 ===========
        for bidx in range(NB):
            tb = tbs[bidx]
            o0, o1 = off[bidx], off[bidx + 1]
            dstl_sb = spool.tile([P, tmax], BF16, tag="dstl")
            nc.sync.dma_start(dstl_sb[:, 0:tb], edstl[:, o0:o1])
            norm_sb = spool.tile([P, tmax], BF16, tag="norm")
            nc.sync.dma_start(norm_sb[:, 0:tb], enorm[:, o0:o1])
            src_sb = spool.tile([P, tmax], I32, tag="srcs")
            nc.sync.dma_start(src_sb[:, 0:tb], esrc[:, o0:o1])

            s_t = spool.tile([P, tmax, P], BF16, tag="s_t")
            nc.vector.tensor_tensor(
                out=s_t[:, 0:tb, :], in0=iota_big[:, 0:tb, :],
                in1=dstl_sb[:, 0:tb].unsqueeze(-1).to_broadcast([P, tb, P]),
                op=mybir.AluOpType.is_equal)
            nc.vector.tensor_tensor(
                out=s_t[:, 0:tb, :], in0=s_t[:, 0:tb, :],
                in1=norm_sb[:, 0:tb].unsqueeze(-1).to_broadcast([P, tb, P]),
                op=mybir.AluOpType.mult)

            g = gpool.tile([P, tmax, D], BF16, tag="g2")
            acc = psA.tile([P, D], F32, tag="acc")
            for t in range(tb):
                nc.gpsimd.indirect_dma_start(
                    out=g[:, t, :], out_offset=None, in_=ag2_out[:],
                    in_offset=bass.IndirectOffsetOnAxis(ap=src_sb[:, t:t + 1], axis=0),
                )
                nc.tensor.matmul(acc[:], lhsT=s_t[:, t, :], rhs=g[:, t, :],
                                 start=(t == 0), stop=(t == tb - 1))
            selft = work.tile([P, D], F32, tag="self2")
            nc.vector.tensor_scalar(out=selft[:], in0=t2_blks[bidx][:],
                                    scalar1=dsq_sb[:, bidx:bidx + 1], scalar2=None,
                                    op0=mybir.AluOpType.mult)
            ng = work.tile([P, D], F32, tag="ng")
            nc.vector.tensor_tensor(out=ng[:], in0=acc[:], in1=selft[:],
                                    op=mybir.AluOpType.add)
            nc.vector.tensor_tensor(out=ng[:], in0=ng[:], in1=b2_bc[:],
                                    op=mybir.AluOpType.add)
            nc.sync.dma_start(ng_p[bidx * P:(bidx + 1) * P, :], ng[:])

        # =========== candidate scoring ===========
        for rt in range(2):
            cl_sb = work.tile([P, KC], I32, tag="cl")
            nc.sync.dma_start(cl_sb[:], cloc[rt])
            cm_sb = work.tile([P, KC], F32, tag="cm")
            nc.sync.dma_start(cm_sb[:], cmask[rt])
            sc_sb = work.tile([P, KC], F32, tag="sc")
            for k in range(KC):
                cg = gpool.tile([P, D], F32, tag="cg")
                nc.gpsimd.indirect_dma_start(
                    out=cg[:], out_offset=None, in_=ng_p[:],
                    in_offset=bass.IndirectOffsetOnAxis(ap=cl_sb[:, k:k + 1], axis=0),
                )
                prod = work.tile([P, D], F32, tag="prod")
                nc.vector.tensor_tensor(out=prod[:], in0=cg[:], in1=q_tiles[rt][:],
                                        op=mybir.AluOpType.mult)
                nc.vector.tensor_reduce(out=sc_sb[:, k:k + 1], in_=prod[:],
                                        axis=mybir.AxisListType.X,
                                        op=mybir.AluOpType.add)
            nc.vector.tensor_tensor(out=sc_sb[:], in0=sc_sb[:], in1=cm_sb[:],
                                    op=mybir.AluOpType.mult)
            nc.sync.dma_start(score[rt], sc_sb[:])

    nc.compile()
    return nc


def _prep(edge_index: np.ndarray):
    """Host-side graph preprocessing: degrees, norms, per-core sorted edge tiles."""
    src = edge_index[0].astype(np.int64)
    dst = edge_index[1].astype(np.int64)
    deg = np.bincount(dst, minlength=N_NODES).astype(np.float64) + 1.0
    dis = 1.0 / np.sqrt(deg)
    norm = (dis[src] * dis[dst]).astype(np.float32)
    dsq = (dis * dis).astype(np.float32)

    owner = dst // NPC
    order = np.argsort(dst, kind="stable")
    src_s, dst_s, norm_s, own_s = src[order], dst[order], norm[order], owner[order]
    # AG row remap for src
    srow = (src_s // NPC) * NPCP + (src_s % NPC)

    # per (core, block) counts; per-block tile count = max over cores
    local = dst_s - own_s * NPC
    blk = local // P
    dstl = (local % P).astype(np.float32)
    key = own_s * NB + blk
    counts = np.bincount(key, minlength=NC * NB).reshape(NC, NB)
    tbs = tuple(int(x) for x in np.ceil(counts.max(axis=0) / P).astype(np.int64))
    off = np.zeros(NB + 1, np.int64)
    np.cumsum(np.array(tbs), out=off[1:])
    tot = int(off[-1])

    esrc = np.zeros((NC, NB), object)
    starts = np.zeros(NC * NB + 1, np.int64)
    np.cumsum(counts.reshape(-1), out=starts[1:])
    esrc_f = np.zeros((NC, P, tot), np.int32)
    edstl_f = np.zeros((NC, P, tot), np.float32)
    enorm_f = np.zeros((NC, P, tot), np.float32)
    for c in range(NC):
        for b in range(NB):
            sta, end = starts[c * NB + b], starts[c * NB + b + 1]
            n = end - sta
            w = tbs[b]
            buf = np.zeros((3, w * P), np.float64)
            buf[0, :n] = srow[sta:end]
            buf[1, :n] = dstl[sta:end]
            buf[2, :n] = norm_s[sta:end]
            # element j -> (tile j//P, partition j%P); store [P, w]
            esrc_f[c, :, off[b]:off[b + 1]] = buf[0].reshape(w, P).T
            edstl_f[c, :, off[b]:off[b + 1]] = buf[1].reshape(w, P).T
            enorm_f[c, :, off[b]:off[b + 1]] = buf[2].reshape(w, P).T
    esrc = esrc_f
    edstl = edstl_f.astype(ml_dtypes.bfloat16)
    enorm = enorm_f.astype(ml_dtypes.bfloat16)

    # dsq transposed per core: [P, NB], entry (p, b) = dsq[c*NPC + b*P + p] (0 for pads)
    dsqt = np.zeros((NC, P, NB), np.float32)
    for c in range(NC):
        v = np.zeros(NPCP, np.float32)
        v[:NPC] = dsq[c * NPC:(c + 1) * NPC]
        dsqt[c] = v.reshape(NB, P).T
    return esrc, edstl, enorm, dsqt, tbs


def kernel(query_embedding, names_embedding, w1, b1, w2, b2,
           sn_w1, sn_b1, bn_gamma, bn_beta, bn_mean, bn_var,
           sn_w2, sn_b2, edge_index, candidates_indices, top_k):
    query_embedding = np.asarray(query_embedding, np.float32)
    names_embedding = np.asarray(names_embedding, np.float32)
    edge_index = np.asarray(edge_index)
    candidates_indices = np.asarray(candidates_indices)

    esrc, edstl, enorm, dsqt, tbs = _prep(edge_index)

    if tbs not in _CACHE:
        _CACHE[tbs] = _build(tbs)
    nc = _CACHE[tbs]

    # candidate partitioning
    cand = candidates_indices.astype(np.int64)  # [256, 20]
    cown = cand // NPC
    clocal = (cand % NPC).astype(np.int32)

    in_maps = []
    for c in range(NC):
        xp = np.zeros((NPCP, F_IN), np.float32)
        xp[:NPC] = names_embedding[c * NPC:(c + 1) * NPC]
        cl = np.where(cown == c, clocal, 0).astype(np.int32).reshape(2, P, KC)
        cm = (cown == c).astype(np.float32).reshape(2, P, KC)
        in_maps.append(dict(
            xp=xp, w1=np.asarray(w1, np.float32), w2=np.asarray(w2, np.float32),
            b1=np.asarray(b1, np.float32).reshape(1, H),
            b2=np.asarray(b2, np.float32).reshape(1, D),
            query=query_embedding,
            snw1=np.asarray(sn_w1, np.float32), snw2=np.asarray(sn_w2, np.float32),
            snb1=np.asarray(sn_b1, np.float32).reshape(1, 256),
            snb2=np.asarray(sn_b2, np.float32).reshape(1, D),
            bng=np.asarray(bn_gamma, np.float32).reshape(1, 256),
            bnb=np.asarray(bn_beta, np.float32).reshape(1, 256),
            bnm=np.asarray(bn_mean, np.float32).reshape(1, 256),
            bnv=np.asarray(bn_var, np.float32).reshape(1, 256),
            esrc=esrc[c], edstl=edstl[c], enorm=enorm[c], dsqt=dsqt[c],
            cloc=cl, cmask=cm,
        ))

    global _LAST_INMAPS
    _LAST_INMAPS = in_maps
    res = run_bass_kernel_spmd(nc, in_maps, core_ids=list(range(NC)), **_RUN_KW)
    _LAST[0] = res
    out = np.zeros((BQ, KC), np.float32)
    for c in range(NC):
        out += res.results[c]["score"].reshape(BQ, KC)

    k = int(top_k) if np.ndim(top_k) == 0 else int(np.asarray(top_k).item())
    return out[:, :k].copy() if k != KC else out
